# revision 36
# baseline (speedup 1.0000x reference)
"""Deriv2 Matern-5/2 kernel for Trainium2 (Bass/Tile), 8 NeuronCores.

out[i,a,j,b] = c^2 * ( A0[i,j] * delta_ab / l_a^2  -  5*fr[i,j] * D[i,j,a] * D[i,j,b] )
  with r[i,j] = ||(X1_i - X2_j)/l||, fr = (5/3) exp(-sqrt5 r), A0 = fr (1 + sqrt5 r),
  D[i,j,a] = (X1[i,a]-X2[j,a]) / l_a^2.

Sharding: X1 rows split across 8 cores (128 rows each); X2/c/l replicated.

Device-side value convention (sign-flipped, il-factored, symmetric-compressed,
bf16). With Gp[i,a,j] = e2[i,j] * k * (X1[i,a]-X2[j,a]) * inv_l[a],
e2 = exp(-sqrt5 r / 2), k = 5c/sqrt3, and At = c^2 * fr * (1+sqrt5 r):
  V[t=(a<b)] = Gp_a * Gp_b          (rows 0..27, a-major)
  V[t=(a,a)] = Gp_a^2 - At          (rows 28..35)
so out[., a, ., b] = -inv_l[a]*inv_l[b] * V[t(a,b)]. The host widens bf16->f32
and applies the -il_a*il_b plane constants while mirroring (a,b)->(b,a).

Per-core layout: SBUF tiles are [128 rows, pair, j] with j innermost so every
DVE tensor_tensor has packed 2-byte last dims on all operands (2x_1p mode),
and the output DMA per j-tile is one fully contiguous bf16 run per partition.

Engines: PE r2 + Gp matmuls (f32); ACT sqrt/exp/copy chain + PSUM->bf16
copies (+ optional diag Square); DVE G, products, A=e*t; Pool diag -= At.
"""

import sys

if "/opt/trn_rl_repo" not in sys.path:
    sys.path.insert(0, "/opt/trn_rl_repo")

import numpy as np

SQRT5 = 2.2360679774997896
NCORES = 8

# ---- schedule knobs (shared by device builder and host operand packing) ----
TILE_SIZES = [128, 256, 256, 256, 128]  # j-tile sizes, sum == m
R_BOUNDS = [0, 128, 1024]  # sqrt-phase slices; tiny slice 0 = fast start
SQUARE_ENGINE = {0: "act", 1: "pool", 2: "pool", 3: "pool", 4: "act"}
SUB_ENGINE = {0: "dve", 1: "dve", 2: "dve", 3: "dve", 4: "dve"}
SPLIT_OUT_DMA = True  # ship upper rows before the diag rows finish

# Stash of the last BassKernelResults (test harness reads exec_time_ns).
LAST_RESULTS = None


def _pairs(d):
    """Device row order: strict-upper a-major, then the d diagonal rows."""
    ps = []
    for a in range(d):
        for b in range(a + 1, d):
            ps.append((a, b))
    ps += [(a, a) for a in range(d)]
    return ps


def _build_nc(n_rows, m, d, c2, inv_l2, safe_sqrt):
    import contextlib
    from concourse import bass, bacc, tile, mybir

    f32 = mybir.dt.float32
    bf16 = mybir.dt.bfloat16
    AF = mybir.ActivationFunctionType
    P = n_rows
    assert P == 128
    sizes = list(TILE_SIZES)
    assert sum(sizes) == m
    NPAIR = d * (d + 1) // 2
    NUP = NPAIR - d  # 28 strict-upper rows come first

    nc = bacc.Bacc("TRN2", target_bir_lowering=False, debug=False, num_devices=NCORES)

    f32r = mybir.dt.float32r
    # smalls pack: [d+2, P + m]: lhs_r2 | rhs_r2
    W = P + m
    smalls = nc.dram_tensor("smalls", [d + 2, W], f32, kind="ExternalInput")
    # Gp matmul operands in float32r (tf32-class): 1 PE cycle/row vs 4 for
    # fp32. The f32r dtype must be carried by the producing DMA end-to-end.
    lhs_dk = nc.dram_tensor("lhs_dk", [d + 1, P], f32r, kind="ExternalInput")
    # rhs for Gp matmuls, columns ordered (tile, a, j_in_tile)
    rhs_dk = nc.dram_tensor("rhs_dk", [d + 1, m * d], f32r, kind="ExternalInput")
    o = nc.dram_tensor("o", [P, NPAIR * m], bf16, kind="ExternalOutput")

    C0 = 5.0 * c2 / 3.0
    C1 = 5.0 * SQRT5 * c2 / 3.0

    from concourse.tile import add_dep_helper

    # The Tile list scheduler may run any ready instruction; pin each engine's
    # program to emission order (which follows the pipeline's dataflow) so
    # later tiles' cheap ops cannot jump ahead of the current tile's chain.
    _last = {}

    def seq(key, inst):
        prev = _last.get(key)
        if prev is not None:
            add_dep_helper(inst.ins, prev.ins, sync=False, reason="pipeline order")
        _last[key] = inst
        return inst

    with tile.TileContext(nc) as tc, contextlib.ExitStack() as ctx:
        consts = ctx.enter_context(tc.tile_pool(name="consts", bufs=1))

        plane = ctx.enter_context(tc.tile_pool(name="plane", bufs=1))
        psum = ctx.enter_context(tc.tile_pool(name="psum", bufs=8, space="PSUM"))
        dpool = ctx.enter_context(tc.tile_pool(name="dpool", bufs=2))
        gpool = ctx.enter_context(tc.tile_pool(name="gpool", bufs=2))
        vpool = ctx.enter_context(tc.tile_pool(name="vpool", bufs=3))

        sm = consts.tile([d + 2, W], f32)
        nc.sync.dma_start(out=sm, in_=smalls.ap())
        l_dk = consts.tile([d + 1, P], f32r)
        nc.sync.dma_start(out=l_dk, in_=lhs_dk.ap())

        l_r2 = sm[:, 0:P]

        rt = plane.tile([P, m], f32)
        e2t = plane.tile([P, m], bf16)
        et = plane.tile([P, m], bf16)
        tt = plane.tile([P, m], bf16)
        At = plane.tile([P, m], bf16)

        # ---- sqrt phase, emitted lazily per R_BOUNDS slice: a tiny first
        # slice lets tile 0 start ~3us earlier at the price of one extra
        # sqrt/exp table-load pair (ACT has slack). ----
        def emit_sqrt_slice(k):
            c0, c1 = R_BOUNDS[k], R_BOUNDS[k + 1]
            for q0 in range(c0, c1, 512):
                q1 = min(q0 + 512, c1)
                ps = psum.tile([P, 512], f32, name="ps")[:, : q1 - q0]
                nc.tensor.matmul(
                    ps, lhsT=l_r2, rhs=sm[:, P + q0 : P + q1], start=True, stop=True
                )
                qsl = slice(q0, q1)
                if safe_sqrt:
                    seq("act", nc.scalar.activation(out=rt[:, qsl], in_=ps, func=AF.Sqrt))
                else:
                    seq("act", nc.scalar.activation(out=rt[:, qsl], in_=ps, func=AF.Relu))
                    seq("act", nc.scalar.activation(
                        out=rt[:, qsl], in_=rt[:, qsl], func=AF.Sqrt
                    ))

        sqrt_done_idx = 0
        sqrt_done = 0

        def need_r(upto):
            nonlocal sqrt_done_idx, sqrt_done
            while sqrt_done < upto:
                emit_sqrt_slice(sqrt_done_idx)
                sqrt_done_idx += 1
                sqrt_done = R_BOUNDS[sqrt_done_idx]

        # ---- exp phase (exp/copy/square table), tile-aligned slices.
        # Only e2 gates the next G; e/t/At/Square are deferred one tile so
        # they fill engine idle time instead of the critical prefix. ----
        def emit_e2_slice(c0, c1):
            need_r(c1)
            sl = slice(c0, c1)
            seq("act", nc.scalar.activation(
                out=e2t[:, sl], in_=rt[:, sl], func=AF.Exp, scale=-SQRT5 / 2.0
            ))

        def emit_et_slice(c0, c1):
            sl = slice(c0, c1)
            seq("act", nc.scalar.activation(out=et[:, sl], in_=rt[:, sl], func=AF.Exp, scale=-SQRT5))
            seq("act", nc.scalar.activation(
                out=tt[:, sl], in_=rt[:, sl], func=AF.Copy, bias=C0, scale=C1
            ))
            seq("dve", nc.vector.tensor_mul(At[:, sl], et[:, sl], tt[:, sl]))

        chain_done = 0

        # whole rhs_dk resident up front (one DMA): the SP ring later carries
        # the upper-row output DMAs, which block SP.SEQ on V completion —
        # input loads must never queue behind them.
        rhs_sb = consts.tile([d + 1, m * d], f32r)
        nc.sync.dma_start(out=rhs_sb, in_=rhs_dk.ap())
        rches = []
        jp = 0
        for tj in sizes:
            rches.append(rhs_sb[:, jp * d : (jp + tj) * d])
            jp += tj

        o_flat = o.ap()
        j0 = 0
        ocol = 0
        deferred = None  # (t, sl, tj, G, V, Vf, ocol) pending diag-path work

        def emit_diag_path(dt, dsl, dtj, dG, dV, dVf, docol):
            emit_et_slice(dsl.start, dsl.stop)
            sq = SQUARE_ENGINE[dt]
            if sq == "act":
                seq("act", nc.scalar.activation(out=dV[:, NUP:, :], in_=dG, func=AF.Square))
            elif sq == "pool":
                seq("pool", nc.gpsimd.tensor_mul(dV[:, NUP:, :], dG, dG))
            else:
                seq("dve", nc.vector.tensor_mul(dV[:, NUP:, :], dG, dG))
            sub_key = SUB_ENGINE[dt]
            sub_eng = nc.vector if sub_key == "dve" else nc.gpsimd
            seq(sub_key if sub_key == "dve" else "pool", sub_eng.tensor_tensor(
                out=dV[:, NUP:, :],
                in0=dV[:, NUP:, :],
                in1=At[:, dsl].unsqueeze(1).broadcast_to([P, d, dtj]),
                op=mybir.AluOpType.subtract,
            ))
            nc.gpsimd.dma_start(
                out=o_flat[:, docol + NUP * dtj : docol + NPAIR * dtj],
                in_=dVf[:, NUP * dtj : NPAIR * dtj],
            )

        for t, tj in enumerate(sizes):
            if chain_done < j0 + tj:
                emit_e2_slice(chain_done, j0 + tj)
                chain_done = j0 + tj
            sl = slice(j0, j0 + tj)
            # Gp for this tile: [P, d*tj] bf16 via matmuls on (a,j) columns
            rch = rches[t]
            Dk = dpool.tile([P, d * 256], bf16, name="Dk")[:, : d * tj]
            for q0 in range(0, d * tj, 512):
                q1 = min(q0 + 512, d * tj)
                ps = psum.tile([P, 512], f32, name="ps")[:, : q1 - q0]
                nc.tensor.matmul(
                    ps, lhsT=l_dk, rhs=rch[:, q0:q1], start=True, stop=True
                )
                seq("act", nc.scalar.copy(out=Dk[:, q0:q1], in_=ps))
            Dk3 = Dk.rearrange("p (a j) -> p a j", a=d)
            # G = e2 * Dk
            Gf = gpool.tile([P, d * 256], bf16, name="G")[:, : d * tj]
            G = Gf.rearrange("p (a j) -> p a j", a=d)
            seq("dve", nc.vector.tensor_mul(
                G, e2t[:, sl].unsqueeze(1).broadcast_to([P, d, tj]), Dk3
            ))
            Vf = vpool.tile([P, NPAIR * 256], bf16, name="V")[:, : NPAIR * tj]
            V = Vf.rearrange("p (r j) -> p r j", r=NPAIR)
            # diag path of the PREVIOUS tile (or of THIS tile when last):
            # emitted before the products so its sub + diag-DMA slot into the
            # engines' slack and the diag rows ship early.
            if deferred is not None:
                emit_diag_path(*deferred)
                deferred = None
            if t == len(sizes) - 1:
                emit_diag_path(t, sl, tj, G, V, Vf, ocol)
            else:
                deferred = (t, sl, tj, G, V, Vf, ocol)
            # strict-upper rows: G_a * G_{a+1..}
            off = 0
            for a in range(d - 1):
                w = d - 1 - a
                seq("dve", nc.vector.tensor_mul(
                    V[:, off : off + w, :],
                    G[:, a, :].unsqueeze(1).broadcast_to([P, w, tj]),
                    G[:, a + 1 :, :],
                ))
                off += w
            # ship the finished upper rows (SP ring: nothing queues behind
            # it except later output DMAs; inputs were prefetched).
            nc.sync.dma_start(
                out=o_flat[:, ocol : ocol + NUP * tj], in_=Vf[:, : NUP * tj]
            )
            j0 += tj
            ocol += NPAIR * tj

    nc.compile()
    return nc


def _host_operands(X1s, X2, inv_l2, l, c2):
    """Per-core matmul operands, host-side (all f32)."""
    P, d = X1s.shape
    m = X2.shape[0]
    inv_l = 1.0 / l
    k = np.sqrt(25.0 * c2 / 3.0)
    ud = X1s.astype(np.float64) / l.astype(np.float64)
    vd = X2.astype(np.float64) / l.astype(np.float64)
    u = ud.astype(np.float32)
    v = vd.astype(np.float32)
    u2 = (ud * ud).sum(1).astype(np.float32)
    v2 = (vd * vd).sum(1).astype(np.float32)
    lhs_r2 = np.concatenate([u.T, u2[None, :], np.ones((1, P), np.float32)], 0)
    rhs_r2 = np.concatenate([-2.0 * v.T, np.ones((1, m), np.float32), v2[None, :]], 0)
    # Gp uses inv_l (NOT inv_l2): the remaining il_a*il_b plane factor is
    # applied on the host during unshard.
    X1il = (X1s * inv_l).astype(np.float32)
    X2il = (X2 * inv_l).astype(np.float32)
    lhs_d = np.concatenate([X1il.T, np.ones((1, P), np.float32)], 0)  # [d+1, P]
    smalls = np.concatenate([lhs_r2, rhs_r2], axis=1)  # [d+2, P+m]
    # rhs_dk columns ordered (tile, a, j_in_tile), variable tile sizes:
    #   row b (b<d): k * delta_{b,a};  row d: -k * X2il[j, a]
    rhs = np.zeros((d + 1, m * d), np.float32)
    j0 = 0
    for tj in TILE_SIZES:
        blk = slice(j0 * d, (j0 + tj) * d)
        for a in range(d):
            rhs[a, blk].reshape(d, tj)[a, :] = k
        rhs[d, blk] = (-k * X2il[j0 : j0 + tj, :].T).reshape(-1)
        j0 += tj
    return {
        "smalls": np.ascontiguousarray(smalls, np.float32),
        "lhs_dk": np.ascontiguousarray(lhs_d, np.float32),
        "rhs_dk": np.ascontiguousarray(rhs, np.float32),
    }


def kernel(X1, X2, c, l):
    global LAST_RESULTS
    from concourse import bass_utils

    X1 = np.ascontiguousarray(np.asarray(X1), dtype=np.float32)
    X2 = np.ascontiguousarray(np.asarray(X2), dtype=np.float32)
    l = np.asarray(l, dtype=np.float32)
    c2 = float(np.asarray(c)) ** 2
    n, d = X1.shape
    m = X2.shape[0]
    assert n % NCORES == 0
    rows = n // NCORES
    NPAIR = d * (d + 1) // 2
    inv_l2 = (1.0 / (l * l)).astype(np.float32)
    inv_l = (1.0 / l).astype(np.float64)

    u = (X1 / l).astype(np.float32)
    v = (X2 / l).astype(np.float32)
    r2_min = float(
        np.min(
            (u * u).sum(1)[:, None]
            + (v * v).sum(1)[None, :]
            - 2.0 * (u @ v.T)
        )
    )
    safe_sqrt = r2_min > 3e-5

    nc = _build_nc(rows, m, d, c2, inv_l2, safe_sqrt)

    in_maps = []
    for core in range(NCORES):
        X1s = X1[core * rows : (core + 1) * rows]
        in_maps.append(_host_operands(X1s, X2, inv_l2, l, c2))

    res = bass_utils.run_bass_kernel_spmd(nc, in_maps, core_ids=list(range(NCORES)))
    LAST_RESULTS = res

    # Host unshard: bf16 -> f32, scale each pair plane by -il_a*il_b, mirror.
    out = np.empty((n, d, m, d), np.float32)
    pairs = _pairs(d)
    scales = [-(float(inv_l[a]) * float(inv_l[b])) for (a, b) in pairs]
    # column layout: per tile a [NPAIR, tj] block
    bounds = np.cumsum([0] + [NPAIR * tj for tj in TILE_SIZES])
    for core in range(NCORES):
        raw = np.asarray(res.results[core]["o"])
        u16 = raw.view(np.uint16).reshape(rows, NPAIR * m)
        f32 = (u16.astype(np.uint32) << 16).view(np.float32)
        Vf = np.empty((rows, NPAIR, m), np.float32)
        j0 = 0
        for ti, tj in enumerate(TILE_SIZES):
            blk = f32[:, bounds[ti] : bounds[ti + 1]].reshape(rows, NPAIR, tj)
            Vf[:, :, j0 : j0 + tj] = blk
            j0 += tj
        r0 = core * rows
        for t, (a, b) in enumerate(pairs):
            plane = Vf[:, t, :] * scales[t]
            out[r0 : r0 + rows, a, :, b] = plane
            if a != b:
                out[r0 : r0 + rows, b, :, a] = plane
    return out


# revision 38
# speedup vs baseline: 1.0582x; 1.0582x over previous
"""Deriv2 Matern-5/2 kernel for Trainium2 (Bass/Tile), 8 NeuronCores.

out[i,a,j,b] = c^2 * ( A0[i,j] * delta_ab / l_a^2  -  5*fr[i,j] * D[i,j,a] * D[i,j,b] )
  with r[i,j] = ||(X1_i - X2_j)/l||, fr = (5/3) exp(-sqrt5 r), A0 = fr (1 + sqrt5 r),
  D[i,j,a] = (X1[i,a]-X2[j,a]) / l_a^2.

Sharding: X1 rows split across 8 cores (128 rows each); X2/c/l replicated.

Device-side value convention (sign-flipped, il-factored, symmetric-compressed,
bf16). With Gp[i,a,j] = e2[i,j] * k * (X1[i,a]-X2[j,a]) * inv_l[a],
e2 = exp(-sqrt5 r / 2), k = 5c/sqrt3, and At = c^2 * fr * (1+sqrt5 r):
  V[t=(a<b)] = Gp_a * Gp_b          (rows 0..27, a-major)
  V[t=(a,a)] = Gp_a^2 - At          (rows 28..35)
so out[., a, ., b] = -inv_l[a]*inv_l[b] * V[t(a,b)]. The host widens bf16->f32
and applies the -il_a*il_b plane constants while mirroring (a,b)->(b,a).

Per-core layout: SBUF tiles are [128 rows, pair, j] with j innermost so every
DVE tensor_tensor has packed 2-byte last dims on all operands (2x_1p mode),
and the output DMA per j-tile is one fully contiguous bf16 run per partition.

Engines: PE r2 + Gp matmuls (f32); ACT sqrt/exp/copy chain + PSUM->bf16
copies (+ optional diag Square); DVE G, products, A=e*t; Pool diag -= At.
"""

import sys

if "/opt/trn_rl_repo" not in sys.path:
    sys.path.insert(0, "/opt/trn_rl_repo")

import numpy as np

SQRT5 = 2.2360679774997896
NCORES = 8

# ---- schedule knobs (shared by device builder and host operand packing) ----
TILE_SIZES = [224, 160, 160, 160, 160, 160]  # j-tile sizes, sum == m
R_BOUNDS = [0, 224, 1024]  # sqrt-phase slices; small slice 0 = fast start
SQUARE_ENGINE = {i: "act" for i in range(6)}
SUB_ENGINE = {i: "pool" for i in range(6)}
SPLIT_OUT_DMA = True  # ship upper rows before the diag rows finish

# Stash of the last BassKernelResults (test harness reads exec_time_ns).
LAST_RESULTS = None


def _pairs(d):
    """Device row order: strict-upper a-major, then the d diagonal rows."""
    ps = []
    for a in range(d):
        for b in range(a + 1, d):
            ps.append((a, b))
    ps += [(a, a) for a in range(d)]
    return ps


def _build_nc(n_rows, m, d, c2, inv_l2, safe_sqrt):
    import contextlib
    from concourse import bass, bacc, tile, mybir

    f32 = mybir.dt.float32
    bf16 = mybir.dt.bfloat16
    AF = mybir.ActivationFunctionType
    P = n_rows
    assert P == 128
    sizes = list(TILE_SIZES)
    assert sum(sizes) == m
    TJMAX = max(sizes)
    NPAIR = d * (d + 1) // 2
    NUP = NPAIR - d  # 28 strict-upper rows come first

    nc = bacc.Bacc("TRN2", target_bir_lowering=False, debug=False, num_devices=NCORES)

    f32r = mybir.dt.float32r
    # smalls pack: [d+2, P + m]: lhs_r2 | rhs_r2
    W = P + m
    smalls = nc.dram_tensor("smalls", [d + 2, W], f32, kind="ExternalInput")
    # Gp matmul operands in float32r (tf32-class): 1 PE cycle/row vs 4 for
    # fp32. The f32r dtype must be carried by the producing DMA end-to-end.
    lhs_dk = nc.dram_tensor("lhs_dk", [d + 1, P], f32r, kind="ExternalInput")
    # rhs for Gp matmuls, columns ordered (tile, a, j_in_tile)
    rhs_dk = nc.dram_tensor("rhs_dk", [d + 1, m * d], f32r, kind="ExternalInput")
    o = nc.dram_tensor("o", [P, NPAIR * m], bf16, kind="ExternalOutput")

    C0 = 5.0 * c2 / 3.0
    C1 = 5.0 * SQRT5 * c2 / 3.0

    from concourse.tile import add_dep_helper

    # The Tile list scheduler may run any ready instruction; pin each engine's
    # program to emission order (which follows the pipeline's dataflow) so
    # later tiles' cheap ops cannot jump ahead of the current tile's chain.
    _last = {}

    def seq(key, inst):
        prev = _last.get(key)
        if prev is not None:
            add_dep_helper(inst.ins, prev.ins, sync=False, reason="pipeline order")
        _last[key] = inst
        return inst

    with tile.TileContext(nc) as tc, contextlib.ExitStack() as ctx:
        consts = ctx.enter_context(tc.tile_pool(name="consts", bufs=1))

        plane = ctx.enter_context(tc.tile_pool(name="plane", bufs=1))
        psum = ctx.enter_context(tc.tile_pool(name="psum", bufs=8, space="PSUM"))
        dpool = ctx.enter_context(tc.tile_pool(name="dpool", bufs=2))
        gpool = ctx.enter_context(tc.tile_pool(name="gpool", bufs=2))
        vpool = ctx.enter_context(tc.tile_pool(name="vpool", bufs=3))

        sm = consts.tile([d + 2, W], f32)
        nc.sync.dma_start(out=sm, in_=smalls.ap())
        l_dk = consts.tile([d + 1, P], f32r)
        nc.sync.dma_start(out=l_dk, in_=lhs_dk.ap())

        l_r2 = sm[:, 0:P]

        rt = plane.tile([P, m], f32)
        e2t = plane.tile([P, m], bf16)
        et = plane.tile([P, m], bf16)
        tt = plane.tile([P, m], bf16)
        At = plane.tile([P, m], bf16)

        # ---- sqrt phase, emitted lazily per R_BOUNDS slice: a tiny first
        # slice lets tile 0 start ~3us earlier at the price of one extra
        # sqrt/exp table-load pair (ACT has slack). ----
        def emit_sqrt_slice(k):
            c0, c1 = R_BOUNDS[k], R_BOUNDS[k + 1]
            for q0 in range(c0, c1, 512):
                q1 = min(q0 + 512, c1)
                ps = psum.tile([P, 512], f32, name="ps")[:, : q1 - q0]
                nc.tensor.matmul(
                    ps, lhsT=l_r2, rhs=sm[:, P + q0 : P + q1], start=True, stop=True
                )
                qsl = slice(q0, q1)
                if safe_sqrt:
                    seq("act", nc.scalar.activation(out=rt[:, qsl], in_=ps, func=AF.Sqrt))
                else:
                    seq("act", nc.scalar.activation(out=rt[:, qsl], in_=ps, func=AF.Relu))
                    seq("act", nc.scalar.activation(
                        out=rt[:, qsl], in_=rt[:, qsl], func=AF.Sqrt
                    ))

        sqrt_done_idx = 0
        sqrt_done = 0

        def need_r(upto):
            nonlocal sqrt_done_idx, sqrt_done
            while sqrt_done < upto:
                emit_sqrt_slice(sqrt_done_idx)
                sqrt_done_idx += 1
                sqrt_done = R_BOUNDS[sqrt_done_idx]

        # ---- exp phase (exp/copy/square table), tile-aligned slices.
        # Only e2 gates the next G; e/t/At/Square are deferred one tile so
        # they fill engine idle time instead of the critical prefix. ----
        def emit_e2_slice(c0, c1):
            need_r(c1)
            sl = slice(c0, c1)
            seq("act", nc.scalar.activation(
                out=e2t[:, sl], in_=rt[:, sl], func=AF.Exp, scale=-SQRT5 / 2.0
            ))

        def emit_et_slice(c0, c1):
            sl = slice(c0, c1)
            seq("act", nc.scalar.activation(out=et[:, sl], in_=rt[:, sl], func=AF.Exp, scale=-SQRT5))
            seq("act", nc.scalar.activation(
                out=tt[:, sl], in_=rt[:, sl], func=AF.Copy, bias=C0, scale=C1
            ))
            seq("dve", nc.vector.tensor_mul(At[:, sl], et[:, sl], tt[:, sl]))

        chain_done = 0

        # whole rhs_dk resident up front (one DMA): the SP ring later carries
        # the upper-row output DMAs, which block SP.SEQ on V completion —
        # input loads must never queue behind them.
        rhs_sb = consts.tile([d + 1, m * d], f32r)
        nc.sync.dma_start(out=rhs_sb, in_=rhs_dk.ap())
        rches = []
        jp = 0
        for tj in sizes:
            rches.append(rhs_sb[:, jp * d : (jp + tj) * d])
            jp += tj

        o_flat = o.ap()
        j0 = 0
        ocol = 0
        deferred = None  # (t, sl, tj, G, V, Vf, ocol) pending diag-path work

        def emit_diag_path(dt, dsl, dtj, dG, dV, dVf, docol):
            emit_et_slice(dsl.start, dsl.stop)
            sq = SQUARE_ENGINE[dt]
            if sq == "act":
                seq("act", nc.scalar.activation(out=dV[:, NUP:, :], in_=dG, func=AF.Square))
            elif sq == "pool":
                seq("pool", nc.gpsimd.tensor_mul(dV[:, NUP:, :], dG, dG))
            else:
                seq("dve", nc.vector.tensor_mul(dV[:, NUP:, :], dG, dG))
            sub_key = SUB_ENGINE[dt]
            sub_eng = nc.vector if sub_key == "dve" else nc.gpsimd
            seq(sub_key if sub_key == "dve" else "pool", sub_eng.tensor_tensor(
                out=dV[:, NUP:, :],
                in0=dV[:, NUP:, :],
                in1=At[:, dsl].unsqueeze(1).broadcast_to([P, d, dtj]),
                op=mybir.AluOpType.subtract,
            ))
            nc.gpsimd.dma_start(
                out=o_flat[:, docol + NUP * dtj : docol + NPAIR * dtj],
                in_=dVf[:, NUP * dtj : NPAIR * dtj],
            )

        for t, tj in enumerate(sizes):
            if chain_done < j0 + tj:
                emit_e2_slice(chain_done, j0 + tj)
                chain_done = j0 + tj
            sl = slice(j0, j0 + tj)
            # Gp for this tile: [P, d*tj] bf16 via matmuls on (a,j) columns
            rch = rches[t]
            Dk = dpool.tile([P, d * TJMAX], bf16, name="Dk")[:, : d * tj]
            for q0 in range(0, d * tj, 512):
                q1 = min(q0 + 512, d * tj)
                ps = psum.tile([P, 512], f32, name="ps")[:, : q1 - q0]
                nc.tensor.matmul(
                    ps, lhsT=l_dk, rhs=rch[:, q0:q1], start=True, stop=True
                )
                seq("act", nc.scalar.copy(out=Dk[:, q0:q1], in_=ps))
            Dk3 = Dk.rearrange("p (a j) -> p a j", a=d)
            # G = e2 * Dk
            Gf = gpool.tile([P, d * TJMAX], bf16, name="G")[:, : d * tj]
            G = Gf.rearrange("p (a j) -> p a j", a=d)
            seq("dve", nc.vector.tensor_mul(
                G, e2t[:, sl].unsqueeze(1).broadcast_to([P, d, tj]), Dk3
            ))
            Vf = vpool.tile([P, NPAIR * TJMAX], bf16, name="V")[:, : NPAIR * tj]
            V = Vf.rearrange("p (r j) -> p r j", r=NPAIR)
            # diag path of the PREVIOUS tile (or of THIS tile when last):
            # emitted before the products so its sub + diag-DMA slot into the
            # engines' slack and the diag rows ship early.
            if deferred is not None:
                emit_diag_path(*deferred)
                deferred = None
            if t == len(sizes) - 1:
                emit_diag_path(t, sl, tj, G, V, Vf, ocol)
            else:
                deferred = (t, sl, tj, G, V, Vf, ocol)
            # strict-upper rows: G_a * G_{a+1..}
            off = 0
            for a in range(d - 1):
                w = d - 1 - a
                seq("dve", nc.vector.tensor_mul(
                    V[:, off : off + w, :],
                    G[:, a, :].unsqueeze(1).broadcast_to([P, w, tj]),
                    G[:, a + 1 :, :],
                ))
                off += w
            # ship the finished upper rows (SP ring: nothing queues behind
            # it except later output DMAs; inputs were prefetched).
            nc.sync.dma_start(
                out=o_flat[:, ocol : ocol + NUP * tj], in_=Vf[:, : NUP * tj]
            )
            j0 += tj
            ocol += NPAIR * tj

    nc.compile()
    return nc


def _host_operands(X1s, X2, inv_l2, l, c2):
    """Per-core matmul operands, host-side (all f32)."""
    P, d = X1s.shape
    m = X2.shape[0]
    inv_l = 1.0 / l
    k = np.sqrt(25.0 * c2 / 3.0)
    ud = X1s.astype(np.float64) / l.astype(np.float64)
    vd = X2.astype(np.float64) / l.astype(np.float64)
    u = ud.astype(np.float32)
    v = vd.astype(np.float32)
    u2 = (ud * ud).sum(1).astype(np.float32)
    v2 = (vd * vd).sum(1).astype(np.float32)
    lhs_r2 = np.concatenate([u.T, u2[None, :], np.ones((1, P), np.float32)], 0)
    rhs_r2 = np.concatenate([-2.0 * v.T, np.ones((1, m), np.float32), v2[None, :]], 0)
    # Gp uses inv_l (NOT inv_l2): the remaining il_a*il_b plane factor is
    # applied on the host during unshard.
    X1il = (X1s * inv_l).astype(np.float32)
    X2il = (X2 * inv_l).astype(np.float32)
    lhs_d = np.concatenate([X1il.T, np.ones((1, P), np.float32)], 0)  # [d+1, P]
    smalls = np.concatenate([lhs_r2, rhs_r2], axis=1)  # [d+2, P+m]
    # rhs_dk columns ordered (tile, a, j_in_tile), variable tile sizes:
    #   row b (b<d): k * delta_{b,a};  row d: -k * X2il[j, a]
    rhs = np.zeros((d + 1, m * d), np.float32)
    j0 = 0
    for tj in TILE_SIZES:
        blk = slice(j0 * d, (j0 + tj) * d)
        for a in range(d):
            rhs[a, blk].reshape(d, tj)[a, :] = k
        rhs[d, blk] = (-k * X2il[j0 : j0 + tj, :].T).reshape(-1)
        j0 += tj
    return {
        "smalls": np.ascontiguousarray(smalls, np.float32),
        "lhs_dk": np.ascontiguousarray(lhs_d, np.float32),
        "rhs_dk": np.ascontiguousarray(rhs, np.float32),
    }


def kernel(X1, X2, c, l):
    global LAST_RESULTS
    from concourse import bass_utils

    X1 = np.ascontiguousarray(np.asarray(X1), dtype=np.float32)
    X2 = np.ascontiguousarray(np.asarray(X2), dtype=np.float32)
    l = np.asarray(l, dtype=np.float32)
    c2 = float(np.asarray(c)) ** 2
    n, d = X1.shape
    m = X2.shape[0]
    assert n % NCORES == 0
    rows = n // NCORES
    NPAIR = d * (d + 1) // 2
    inv_l2 = (1.0 / (l * l)).astype(np.float32)
    inv_l = (1.0 / l).astype(np.float64)

    u = (X1 / l).astype(np.float32)
    v = (X2 / l).astype(np.float32)
    r2_min = float(
        np.min(
            (u * u).sum(1)[:, None]
            + (v * v).sum(1)[None, :]
            - 2.0 * (u @ v.T)
        )
    )
    safe_sqrt = r2_min > 3e-5

    nc = _build_nc(rows, m, d, c2, inv_l2, safe_sqrt)

    in_maps = []
    for core in range(NCORES):
        X1s = X1[core * rows : (core + 1) * rows]
        in_maps.append(_host_operands(X1s, X2, inv_l2, l, c2))

    res = bass_utils.run_bass_kernel_spmd(nc, in_maps, core_ids=list(range(NCORES)))
    LAST_RESULTS = res

    # Host unshard: bf16 -> f32, scale each pair plane by -il_a*il_b, mirror.
    out = np.empty((n, d, m, d), np.float32)
    pairs = _pairs(d)
    scales = [-(float(inv_l[a]) * float(inv_l[b])) for (a, b) in pairs]
    # column layout: per tile a [NPAIR, tj] block
    bounds = np.cumsum([0] + [NPAIR * tj for tj in TILE_SIZES])
    for core in range(NCORES):
        raw = np.asarray(res.results[core]["o"])
        u16 = raw.view(np.uint16).reshape(rows, NPAIR * m)
        f32 = (u16.astype(np.uint32) << 16).view(np.float32)
        Vf = np.empty((rows, NPAIR, m), np.float32)
        j0 = 0
        for ti, tj in enumerate(TILE_SIZES):
            blk = f32[:, bounds[ti] : bounds[ti + 1]].reshape(rows, NPAIR, tj)
            Vf[:, :, j0 : j0 + tj] = blk
            j0 += tj
        r0 = core * rows
        for t, (a, b) in enumerate(pairs):
            plane = Vf[:, t, :] * scales[t]
            out[r0 : r0 + rows, a, :, b] = plane
            if a != b:
                out[r0 : r0 + rows, b, :, a] = plane
    return out
